# revision 22
# baseline (speedup 1.0000x reference)
"""Trainium2 Bass kernel for the Doppler channel problem.

Math (per batch row n, antenna p):
    weight[n,p,s] = sum_l cof[n,p,l] * shift[l,s]          (complex, L=16, S=14)
    out[n,p,s,k]  = x[n,p,s,k] * weight[n,p,s]             (broadcast over MK=80)
    H_t[n,p,m]    = sum_l cof[n,p,l] * F[l,m]              (64-pt DFT of zero-padded taps)

Sharding: pure data parallelism over the batch dim N (1024) across 8 cores;
each core handles 128 batch rows = 1024 (n,p) rows. No cross-core comms.

Device layout per core: (n,p) rows go to SBUF partitions, 2 rows per
partition (chunk of 256 rows -> one (128, 2240) tile, 4 chunks per core).
The tiny complex matmuls (weight, H_t) run on the tensor engine with the
stacked-[cof_r; cof_i] trick: one fp32 matmul per (chunk, row-parity) with a
host-provided constant rhs produces [w_r | -w_i | H_r | H_i] in PSUM.
The broadcast multiply runs on the vector engine as per-(row,s) 2x-mode
tensor_scalar products plus two full-width combines.
"""

import numpy as np

import concourse.bass as bass
import concourse.tile as tile
from concourse import bacc, mybir
from concourse.bass_utils import run_bass_kernel_spmd

# Problem constants (hardcoded per spec nn_Channel_86947317940845)
L = 16
M = 64
N_PILOT = 2
V = 100.0
N, P, SMK = 1024, 8, 1120
NC = 8          # cores
RPC = N // NC   # batch rows per core (128)
ROWS = RPC * P  # (n,p) rows per core (1024)
G = 2           # (n,p) rows per partition
CHUNK = 128 * G      # rows per chunk (256)
NCHUNK = ROWS // CHUNK  # 4
FD = G * SMK         # free dim per tile (2240)

_F32 = mybir.dt.float32


def _constants(S, MK):
    """Host-side constant matrices: rhs for the PE matmul.

    rhs_all (2L, 2S + 2M) fp32, for stacked lhsT = [cof_r; cof_i]:
      cols [0:S)        -> w_r    = cof_r@s_r - cof_i@s_i   : [s_r; -s_i]
      cols [S:2S)       -> -w_i   = -(cof_r@s_i + cof_i@s_r): [-s_i; -s_r]
      cols [2S:2S+M)    -> H_r    = cof_r@F_r - cof_i@F_i   : [F_r; -F_i]
      cols [2S+M:2S+2M) -> H_i    = cof_r@F_i + cof_i@F_r   : [F_i; F_r]
    """
    t = np.linspace(0.0, (S - 1) * (0.0005 / 14.0), S)
    fd_max = V / 3.0e8 * 3.0e9
    angles = np.linspace(0.0, 2.0 * np.pi, L)
    phases = np.outer(2.0 * np.pi * np.cos(angles) * fd_max, t)  # (L,S)
    sr, si = np.cos(phases), np.sin(phases)
    lm = np.outer(np.arange(L), np.arange(M)) * (2.0 * np.pi / M)
    fr, fi = np.cos(lm), -np.sin(lm)  # F = exp(-2i pi l m / M)
    top = np.concatenate([sr, -si, fr, fi], axis=1)
    bot = np.concatenate([-si, -sr, -fi, fr], axis=1)
    return np.concatenate([top, bot], axis=0).astype(np.float32)  # (32, 2S+2M)


def _build(S, MK, repeats=1, gps_units=0, g_rows=G, dma_split=False,
           chunk_plan=None, gps_s=None, split_out=False, xs_bufs=3, act_s=0):
    """chunk_plan: list of per-chunk row-group counts g (sum(g)*128 == ROWS).
    gps_s: if set, units with s < gps_s (within each g) go to GPSIMD and
    gps_units is ignored. split_out: one output DMA per row-group instead of
    per chunk (starts stores earlier)."""
    RHS_W = 2 * S          # 28
    RHS_ALL = 2 * S + 2 * M  # 156
    HW = 2 * M             # H row width per (n,p) row (128)
    if chunk_plan is None:
        chunk_plan = [g_rows] * (ROWS // (128 * g_rows))
    assert sum(chunk_plan) * 128 == ROWS
    GMAX = max(chunk_plan)

    nc = bacc.Bacc("TRN2", target_bir_lowering=False, debug=False, num_devices=NC)
    xr = nc.dram_tensor("xr", [ROWS, SMK], _F32, kind="ExternalInput").ap()
    xi = nc.dram_tensor("xi", [ROWS, SMK], _F32, kind="ExternalInput").ap()
    ct = nc.dram_tensor("ct", [2 * L, ROWS], _F32, kind="ExternalInput").ap()
    rhs = nc.dram_tensor("rhs", [2 * L, RHS_ALL], _F32, kind="ExternalInput").ap()
    our = nc.dram_tensor("our", [ROWS, SMK], _F32, kind="ExternalOutput").ap()
    oui = nc.dram_tensor("oui", [ROWS, SMK], _F32, kind="ExternalOutput").ap()
    ht = nc.dram_tensor("ht", [ROWS, HW], _F32, kind="ExternalOutput").ap()

    # per-chunk views: partition p of chunk at row r0 (gg rows/partition)
    # holds rows r0 + gg*p + g, g in [0, gg)
    def view(t, r0, gg, w):
        return t[r0 : r0 + 128 * gg, :].rearrange("(p a) m -> p (a m)", a=gg)

    chunks = []  # (row0, gg, lhsT column block start)
    r0 = 0
    blk = 0
    for gg in chunk_plan:
        chunks.append((r0, gg, blk))
        r0 += 128 * gg
        blk += gg

    dma_out = nc.scalar.dma_start if dma_split else nc.sync.dma_start
    with tile.TileContext(nc) as tc:
        with (
            tc.tile_pool(name="consts", bufs=1) as consts,
            tc.tile_pool(name="xs", bufs=xs_bufs) as xs,
            tc.tile_pool(name="ts", bufs=2) as tpool,
            tc.tile_pool(name="os", bufs=2) as opool,
            tc.tile_pool(name="ws", bufs=2) as wpool,
            tc.tile_pool(name="hs", bufs=2) as hpool,
            tc.tile_pool(name="psum", bufs=4, space="PSUM") as psum,
        ):
            ct_sb = consts.tile([2 * L, ROWS], _F32)
            nc.sync.dma_start(out=ct_sb[:], in_=ct[:])
            rhs_sb = consts.tile([2 * L, RHS_ALL], _F32)
            nc.sync.dma_start(out=rhs_sb[:], in_=rhs[:])

            for r0, gg, blk in chunks * repeats:
                fdl = gg * SMK
                xr_t = xs.tile([128, GMAX * SMK], _F32, tag="xr")
                nc.sync.dma_start(out=xr_t[:, :fdl], in_=view(xr, r0, gg, SMK))
                xi_t = xs.tile([128, GMAX * SMK], _F32, tag="xi")
                nc.sync.dma_start(out=xi_t[:, :fdl], in_=view(xi, r0, gg, SMK))

                w_t = wpool.tile([128, GMAX * RHS_W], _F32)
                h_t = hpool.tile([128, GMAX * HW], _F32)
                for g in range(gg):
                    pw = psum.tile([128, RHS_ALL], _F32, tag="pw")
                    nc.tensor.matmul(
                        pw[:],
                        ct_sb[:, (blk + g) * 128 : (blk + g + 1) * 128],
                        rhs_sb[:],
                        start=True,
                        stop=True,
                    )
                    nc.scalar.copy(
                        w_t[:, g * RHS_W : (g + 1) * RHS_W], pw[:, 0:RHS_W]
                    )
                    nc.scalar.copy(
                        h_t[:, g * HW : (g + 1) * HW], pw[:, RHS_W : RHS_W + HW]
                    )
                dma_out(out=view(ht, r0, gg, HW), in_=h_t[:, : gg * HW])

                # per (g,s) unit: with win = -w_i,
                #   a = xr*win = -xr*wi ; out_i = (xi*wr) - a = xi*wr + xr*wi
                #   b = xi*win = -xi*wi ; out_r = (xr*wr) + b = xr*wr - xi*wi
                ta = tpool.tile([128, GMAX * SMK], _F32, tag="ta")
                tb = tpool.tile([128, GMAX * SMK], _F32, tag="tb")
                tcg = tpool.tile([128, GMAX * SMK], _F32, tag="tc")
                tdg = tpool.tile([128, GMAX * SMK], _F32, tag="td")
                our_t = opool.tile([128, GMAX * SMK], _F32, tag="our")
                oui_t = opool.tile([128, GMAX * SMK], _F32, tag="oui")
                mul = mybir.AluOpType.mult
                our_vv = view(our, r0, gg, SMK)
                oui_vv = view(oui, r0, gg, SMK)
                for g in range(gg):
                    for s in range(S):
                        if gps_s is not None:
                            e = nc.gpsimd if s < gps_s else nc.vector
                        else:
                            e = nc.gpsimd if g * S + s < gps_units else nc.vector
                        sl = slice(g * SMK + s * MK, g * SMK + (s + 1) * MK)
                        wr = w_t[:, g * RHS_W + s : g * RHS_W + s + 1]
                        win = w_t[:, g * RHS_W + S + s : g * RHS_W + S + s + 1]
                        if e is nc.vector and gps_s is not None and s < gps_s + act_s:
                            # products on the scalar engine (act copy w/ scale)
                            nc.scalar.mul(ta[:, sl], xr_t[:, sl], win)
                            nc.scalar.mul(tb[:, sl], xi_t[:, sl], win)
                        else:
                            e.tensor_scalar_mul(ta[:, sl], xr_t[:, sl], win)
                            e.tensor_scalar_mul(tb[:, sl], xi_t[:, sl], win)
                        if e is nc.vector:
                            # fused: out_i = (xi*wr) - (-xr*wi); out_r = (xr*wr) + (-xi*wi)
                            e.scalar_tensor_tensor(
                                oui_t[:, sl], xi_t[:, sl], wr, ta[:, sl],
                                mul, mybir.AluOpType.subtract,
                            )
                            e.scalar_tensor_tensor(
                                our_t[:, sl], xr_t[:, sl], wr, tb[:, sl],
                                mul, mybir.AluOpType.add,
                            )
                        else:
                            # walrus has no Pool-engine STT; use TS products + TT
                            e.tensor_scalar_mul(tcg[:, sl], xi_t[:, sl], wr)
                            e.tensor_scalar_mul(tdg[:, sl], xr_t[:, sl], wr)
                            e.tensor_tensor(
                                oui_t[:, sl], tcg[:, sl], ta[:, sl],
                                mybir.AluOpType.subtract,
                            )
                            e.tensor_tensor(
                                our_t[:, sl], tdg[:, sl], tb[:, sl],
                                mybir.AluOpType.add,
                            )
                    if split_out:
                        gsl = slice(g * SMK, (g + 1) * SMK)
                        dma_out(out=our_vv[:, gsl], in_=our_t[:, gsl])
                        dma_out(out=oui_vv[:, gsl], in_=oui_t[:, gsl])
                if not split_out:
                    dma_out(out=our_vv, in_=our_t[:, :fdl])
                    dma_out(out=oui_vv, in_=oui_t[:, :fdl])

    nc.compile()
    return nc


_CACHE = {}
# tuned config: small head/tail chunks shrink pipeline ramp; s<GPS_S units run
# on GPSIMD (engine balance); outputs DMA per row-group via the ACT HWDGE ring
CHUNK_PLAN = [1, 2, 2, 2, 1]
GPS_S = 4
ACT_S = 3
DMA_SPLIT = True
SPLIT_OUT = True


def _get_nc(S, MK):
    key = (S, MK)
    if key not in _CACHE:
        _CACHE[key] = _build(
            S, MK, chunk_plan=CHUNK_PLAN, gps_s=GPS_S, act_s=ACT_S,
            dma_split=DMA_SPLIT, split_out=SPLIT_OUT,
        )
    return _CACHE[key]


def _chunk_perm(plan):
    perm = []
    r0 = 0
    for gg in plan:
        for g in range(gg):
            perm.extend((r0 + gg * np.arange(128) + g).tolist())
        r0 += 128 * gg
    return np.array(perm)


def _in_maps(input_real, input_imag, cof_real, cof_imag, S, MK, plan=None):
    rhs = _constants(S, MK)
    maps = []
    for c in range(NC):
        sl = slice(c * RPC, (c + 1) * RPC)
        xr = np.ascontiguousarray(input_real[sl]).reshape(ROWS, SMK)
        xi = np.ascontiguousarray(input_imag[sl]).reshape(ROWS, SMK)
        cr = np.ascontiguousarray(cof_real[sl]).reshape(ROWS, L)
        ci = np.ascontiguousarray(cof_imag[sl]).reshape(ROWS, L)
        cs = np.concatenate([cr.T, ci.T], axis=0)  # (2L, ROWS)
        # permute columns so block (G*i+g) holds rows i*CHUNK + G*p + g
        if plan is None:
            plan = CHUNK_PLAN
        cs = np.ascontiguousarray(cs[:, _chunk_perm(plan)])
        maps.append({"xr": xr, "xi": xi, "ct": cs, "rhs": rhs})
    return maps


def kernel(input_real, input_imag, cof_real, cof_imag, Ns):
    S = int(Ns) + N_PILOT
    MK = SMK // S
    assert S * MK == SMK and S == 14

    nc = _get_nc(S, MK)
    maps = _in_maps(input_real, input_imag, cof_real, cof_imag, S, MK)
    res = run_bass_kernel_spmd(nc, maps, core_ids=list(range(NC)))

    out = np.empty((N, P, SMK), dtype=np.complex64)
    H_t = np.empty((N, P, M), dtype=np.complex64)
    for c in range(NC):
        r = res.results[c]
        sl = slice(c * RPC, (c + 1) * RPC)
        out[sl] = (r["our"] + 1j * r["oui"]).reshape(RPC, P, SMK)
        hh = r["ht"].reshape(RPC, P, 2 * M)
        H_t[sl] = hh[..., :M] + 1j * hh[..., M:]
    return out, H_t


# revision 23
# speedup vs baseline: 88233.0661x; 88233.0661x over previous
"""Trainium2 Bass kernel for the Doppler channel problem.

Math (per batch row n, antenna p):
    weight[n,p,s] = sum_l cof[n,p,l] * shift[l,s]          (complex, L=16, S=14)
    out[n,p,s,k]  = x[n,p,s,k] * weight[n,p,s]             (broadcast over MK=80)
    H_t[n,p,m]    = sum_l cof[n,p,l] * F[l,m]              (64-pt DFT of zero-padded taps)

Sharding: pure data parallelism over the batch dim N (1024) across 8 cores;
each core handles 128 batch rows = 1024 (n,p) rows. No cross-core comms.

Device layout per core: (n,p) rows go to SBUF partitions, 2 rows per
partition (chunk of 256 rows -> one (128, 2240) tile, 4 chunks per core).
The tiny complex matmuls (weight, H_t) run on the tensor engine with the
stacked-[cof_r; cof_i] trick: one fp32 matmul per (chunk, row-parity) with a
host-provided constant rhs produces [w_r | -w_i | H_r | H_i] in PSUM.
The broadcast multiply runs on the vector engine as per-(row,s) 2x-mode
tensor_scalar products plus two full-width combines.
"""

import numpy as np

import concourse.bass as bass
import concourse.tile as tile
from concourse import bacc, mybir
from concourse.bass_utils import run_bass_kernel_spmd

# Problem constants (hardcoded per spec nn_Channel_86947317940845)
L = 16
M = 64
N_PILOT = 2
V = 100.0
N, P, SMK = 1024, 8, 1120
NC = 8          # cores
RPC = N // NC   # batch rows per core (128)
ROWS = RPC * P  # (n,p) rows per core (1024)
G = 2           # (n,p) rows per partition
CHUNK = 128 * G      # rows per chunk (256)
NCHUNK = ROWS // CHUNK  # 4
FD = G * SMK         # free dim per tile (2240)

_F32 = mybir.dt.float32


def _constants(S, MK):
    """Host-side constant matrices: rhs for the PE matmul.

    rhs_all (2L, 2S + 2M) fp32, for stacked lhsT = [cof_r; cof_i]:
      cols [0:S)        -> w_r    = cof_r@s_r - cof_i@s_i   : [s_r; -s_i]
      cols [S:2S)       -> -w_i   = -(cof_r@s_i + cof_i@s_r): [-s_i; -s_r]
      cols [2S:2S+M)    -> H_r    = cof_r@F_r - cof_i@F_i   : [F_r; -F_i]
      cols [2S+M:2S+2M) -> H_i    = cof_r@F_i + cof_i@F_r   : [F_i; F_r]
    """
    t = np.linspace(0.0, (S - 1) * (0.0005 / 14.0), S)
    fd_max = V / 3.0e8 * 3.0e9
    angles = np.linspace(0.0, 2.0 * np.pi, L)
    phases = np.outer(2.0 * np.pi * np.cos(angles) * fd_max, t)  # (L,S)
    sr, si = np.cos(phases), np.sin(phases)
    lm = np.outer(np.arange(L), np.arange(M)) * (2.0 * np.pi / M)
    fr, fi = np.cos(lm), -np.sin(lm)  # F = exp(-2i pi l m / M)
    top = np.concatenate([sr, -si, fr, fi], axis=1)
    bot = np.concatenate([-si, -sr, -fi, fr], axis=1)
    return np.concatenate([top, bot], axis=0).astype(np.float32)  # (32, 2S+2M)


def _build(S, MK, repeats=1, gps_units=0, g_rows=G, dma_split=False,
           chunk_plan=None, gps_s=None, split_out=False, xs_bufs=3, act_s=0):
    """chunk_plan: list of per-chunk row-group counts g (sum(g)*128 == ROWS).
    gps_s: if set, units with s < gps_s (within each g) go to GPSIMD and
    gps_units is ignored. split_out: one output DMA per row-group instead of
    per chunk (starts stores earlier)."""
    RHS_W = 2 * S          # 28
    RHS_ALL = 2 * S + 2 * M  # 156
    HW = 2 * M             # H row width per (n,p) row (128)
    if chunk_plan is None:
        chunk_plan = [g_rows] * (ROWS // (128 * g_rows))
    assert sum(chunk_plan) * 128 == ROWS
    GMAX = max(chunk_plan)

    nc = bacc.Bacc("TRN2", target_bir_lowering=False, debug=False, num_devices=NC)
    xr = nc.dram_tensor("xr", [ROWS, SMK], _F32, kind="ExternalInput").ap()
    xi = nc.dram_tensor("xi", [ROWS, SMK], _F32, kind="ExternalInput").ap()
    ct = nc.dram_tensor("ct", [2 * L, ROWS], _F32, kind="ExternalInput").ap()
    rhs = nc.dram_tensor("rhs", [2 * L, RHS_ALL], _F32, kind="ExternalInput").ap()
    our = nc.dram_tensor("our", [ROWS, SMK], _F32, kind="ExternalOutput").ap()
    oui = nc.dram_tensor("oui", [ROWS, SMK], _F32, kind="ExternalOutput").ap()
    ht = nc.dram_tensor("ht", [ROWS, HW], _F32, kind="ExternalOutput").ap()

    # per-chunk views: partition p of chunk at row r0 (gg rows/partition)
    # holds rows r0 + gg*p + g, g in [0, gg)
    def view(t, r0, gg, w):
        return t[r0 : r0 + 128 * gg, :].rearrange("(p a) m -> p (a m)", a=gg)

    chunks = []  # (row0, gg, lhsT column block start)
    r0 = 0
    blk = 0
    for gg in chunk_plan:
        chunks.append((r0, gg, blk))
        r0 += 128 * gg
        blk += gg

    dma_out = nc.scalar.dma_start if dma_split else nc.sync.dma_start
    with tile.TileContext(nc) as tc:
        with (
            tc.tile_pool(name="consts", bufs=1) as consts,
            tc.tile_pool(name="xs", bufs=xs_bufs) as xs,
            tc.tile_pool(name="ts", bufs=2) as tpool,
            tc.tile_pool(name="os", bufs=2) as opool,
            tc.tile_pool(name="ws", bufs=2) as wpool,
            tc.tile_pool(name="hs", bufs=2) as hpool,
            tc.tile_pool(name="psum", bufs=4, space="PSUM") as psum,
        ):
            ct_sb = consts.tile([2 * L, ROWS], _F32)
            nc.sync.dma_start(out=ct_sb[:], in_=ct[:])
            rhs_sb = consts.tile([2 * L, RHS_ALL], _F32)
            nc.sync.dma_start(out=rhs_sb[:], in_=rhs[:])

            for r0, gg, blk in chunks * repeats:
                fdl = gg * SMK
                xr_t = xs.tile([128, GMAX * SMK], _F32, tag="xr")
                nc.sync.dma_start(out=xr_t[:, :fdl], in_=view(xr, r0, gg, SMK))
                xi_t = xs.tile([128, GMAX * SMK], _F32, tag="xi")
                nc.sync.dma_start(out=xi_t[:, :fdl], in_=view(xi, r0, gg, SMK))

                w_t = wpool.tile([128, GMAX * RHS_W], _F32)
                h_t = hpool.tile([128, GMAX * HW], _F32)
                for g in range(gg):
                    pw = psum.tile([128, RHS_ALL], _F32, tag="pw")
                    nc.tensor.matmul(
                        pw[:],
                        ct_sb[:, (blk + g) * 128 : (blk + g + 1) * 128],
                        rhs_sb[:],
                        start=True,
                        stop=True,
                    )
                    nc.scalar.copy(
                        w_t[:, g * RHS_W : (g + 1) * RHS_W], pw[:, 0:RHS_W]
                    )
                    nc.scalar.copy(
                        h_t[:, g * HW : (g + 1) * HW], pw[:, RHS_W : RHS_W + HW]
                    )
                dma_out(out=view(ht, r0, gg, HW), in_=h_t[:, : gg * HW])

                # per (g,s) unit: with win = -w_i,
                #   a = xr*win = -xr*wi ; out_i = (xi*wr) - a = xi*wr + xr*wi
                #   b = xi*win = -xi*wi ; out_r = (xr*wr) + b = xr*wr - xi*wi
                ta = tpool.tile([128, GMAX * SMK], _F32, tag="ta")
                tb = tpool.tile([128, GMAX * SMK], _F32, tag="tb")
                tcg = tpool.tile([128, GMAX * SMK], _F32, tag="tc")
                tdg = tpool.tile([128, GMAX * SMK], _F32, tag="td")
                our_t = opool.tile([128, GMAX * SMK], _F32, tag="our")
                oui_t = opool.tile([128, GMAX * SMK], _F32, tag="oui")
                mul = mybir.AluOpType.mult
                our_vv = view(our, r0, gg, SMK)
                oui_vv = view(oui, r0, gg, SMK)
                for g in range(gg):
                    for s in range(S):
                        if gps_s is not None:
                            e = nc.gpsimd if s < gps_s else nc.vector
                        else:
                            e = nc.gpsimd if g * S + s < gps_units else nc.vector
                        sl = slice(g * SMK + s * MK, g * SMK + (s + 1) * MK)
                        wr = w_t[:, g * RHS_W + s : g * RHS_W + s + 1]
                        win = w_t[:, g * RHS_W + S + s : g * RHS_W + S + s + 1]
                        if e is nc.vector and gps_s is not None and s < gps_s + act_s:
                            # products on the scalar engine (act copy w/ scale)
                            nc.scalar.mul(ta[:, sl], xr_t[:, sl], win)
                            nc.scalar.mul(tb[:, sl], xi_t[:, sl], win)
                        else:
                            e.tensor_scalar_mul(ta[:, sl], xr_t[:, sl], win)
                            e.tensor_scalar_mul(tb[:, sl], xi_t[:, sl], win)
                        if e is nc.vector:
                            # fused: out_i = (xi*wr) - (-xr*wi); out_r = (xr*wr) + (-xi*wi)
                            e.scalar_tensor_tensor(
                                oui_t[:, sl], xi_t[:, sl], wr, ta[:, sl],
                                mul, mybir.AluOpType.subtract,
                            )
                            e.scalar_tensor_tensor(
                                our_t[:, sl], xr_t[:, sl], wr, tb[:, sl],
                                mul, mybir.AluOpType.add,
                            )
                        else:
                            # walrus has no Pool-engine STT; use TS products + TT
                            e.tensor_scalar_mul(tcg[:, sl], xi_t[:, sl], wr)
                            e.tensor_scalar_mul(tdg[:, sl], xr_t[:, sl], wr)
                            e.tensor_tensor(
                                oui_t[:, sl], tcg[:, sl], ta[:, sl],
                                mybir.AluOpType.subtract,
                            )
                            e.tensor_tensor(
                                our_t[:, sl], tdg[:, sl], tb[:, sl],
                                mybir.AluOpType.add,
                            )
                    if split_out:
                        gsl = slice(g * SMK, (g + 1) * SMK)
                        dma_out(out=our_vv[:, gsl], in_=our_t[:, gsl])
                        dma_out(out=oui_vv[:, gsl], in_=oui_t[:, gsl])
                if not split_out:
                    dma_out(out=our_vv, in_=our_t[:, :fdl])
                    dma_out(out=oui_vv, in_=oui_t[:, :fdl])

    nc.compile()
    return nc


_CACHE = {}
# tuned config: small head/tail chunks shrink pipeline ramp; s<GPS_S units run
# on GPSIMD (engine balance); outputs DMA per row-group via the ACT HWDGE ring
CHUNK_PLAN = [1, 2, 2, 2, 1]
GPS_S = 3
ACT_S = 4
DMA_SPLIT = True
SPLIT_OUT = True


def _get_nc(S, MK):
    key = (S, MK)
    if key not in _CACHE:
        _CACHE[key] = _build(
            S, MK, chunk_plan=CHUNK_PLAN, gps_s=GPS_S, act_s=ACT_S,
            dma_split=DMA_SPLIT, split_out=SPLIT_OUT,
        )
    return _CACHE[key]


def _chunk_perm(plan):
    perm = []
    r0 = 0
    for gg in plan:
        for g in range(gg):
            perm.extend((r0 + gg * np.arange(128) + g).tolist())
        r0 += 128 * gg
    return np.array(perm)


def _in_maps(input_real, input_imag, cof_real, cof_imag, S, MK, plan=None):
    rhs = _constants(S, MK)
    maps = []
    for c in range(NC):
        sl = slice(c * RPC, (c + 1) * RPC)
        xr = np.ascontiguousarray(input_real[sl]).reshape(ROWS, SMK)
        xi = np.ascontiguousarray(input_imag[sl]).reshape(ROWS, SMK)
        cr = np.ascontiguousarray(cof_real[sl]).reshape(ROWS, L)
        ci = np.ascontiguousarray(cof_imag[sl]).reshape(ROWS, L)
        cs = np.concatenate([cr.T, ci.T], axis=0)  # (2L, ROWS)
        # permute columns so block (G*i+g) holds rows i*CHUNK + G*p + g
        if plan is None:
            plan = CHUNK_PLAN
        cs = np.ascontiguousarray(cs[:, _chunk_perm(plan)])
        maps.append({"xr": xr, "xi": xi, "ct": cs, "rhs": rhs})
    return maps


def kernel(input_real, input_imag, cof_real, cof_imag, Ns):
    S = int(Ns) + N_PILOT
    MK = SMK // S
    assert S * MK == SMK and S == 14

    nc = _get_nc(S, MK)
    maps = _in_maps(input_real, input_imag, cof_real, cof_imag, S, MK)
    res = run_bass_kernel_spmd(nc, maps, core_ids=list(range(NC)))

    out = np.empty((N, P, SMK), dtype=np.complex64)
    H_t = np.empty((N, P, M), dtype=np.complex64)
    for c in range(NC):
        r = res.results[c]
        sl = slice(c * RPC, (c + 1) * RPC)
        out[sl] = (r["our"] + 1j * r["oui"]).reshape(RPC, P, SMK)
        hh = r["ht"].reshape(RPC, P, 2 * M)
        H_t[sl] = hh[..., :M] + 1j * hh[..., M:]
    return out, H_t
